# revision 1
# baseline (speedup 1.0000x reference)
"""Trainium2 Bass kernel for SimCLR NT-Xent contrastive loss (N=4096, D=512, T=0.5).

Math: with z = rownorm(concat(emb_i, emb_j)) (8192x512) and S = z @ z.T:
  loss = (1/2N) * [ sum_r log(rowsum_r(exp(S/T)) - exp(1/T)) - (1/T) * sum_r S[r, (r+N) mod 2N] ]

Distribution: data-parallel over rows of z. Each of the 8 cores receives a
block-rotated copy of the concatenated input (rotation by 1024*c rows), so the
same program computes row block [0:1024) of its rotated similarity matrix
against all 8192 columns. Rotation preserves both the row set (each original
row handled exactly once across cores) and the +N pair structure (mod 2N).

Per-core pipeline (all compute on device):
  1. load raw f32 rows -> SBUF
  2. rownorm: square+row-sum (DVE, fused accum) -> rsqrt via Quake seed + 2
     Newton steps (DVE only; avoids ACT sqrt<->exp table thrash)
  3. scale rows by 1/norm, cast bf16
  4. bf16 z roundtrip through DRAM + xbar DMA-transpose -> zT [d, rows] in SBUF
  5. 512x bf16 matmul (128x128x512) into PSUM; ACT exp(2x) with fused row-sum
  6. log(denom) via single Ln at the end; positives via fused mul+row-sum on rows
Host merges 8 partial [128,8] tensors (log-denoms, pair-dots) into the scalar.
"""

import numpy as np

for _p in ("/opt/trn_rl_repo", "/root/.axon_site/_ro/trn_rl_repo"):
    try:
        import concourse  # noqa: F401
        break
    except ImportError:
        import sys
        if _p not in sys.path:
            sys.path.insert(0, _p)

import concourse.bass as bass
import concourse.bacc as bacc
import concourse.tile as tile
from concourse import mybir
from concourse.bass_utils import run_bass_kernel_spmd

F32 = mybir.dt.float32
I32 = mybir.dt.int32
BF16 = mybir.dt.bfloat16
ALU = mybir.AluOpType
AF = mybir.ActivationFunctionType

N_CORES = 8
BATCH = 4096
DIM = 512
ROWS = 2 * BATCH            # 8192
BLOCK = ROWS // N_CORES     # 1024 rows per core
P = 128                     # partitions
NT = ROWS // P              # 64 row tiles
NG = 16                     # load groups (4 row-tiles each)
TPG = 4                     # tiles per group
RG = 4                      # DRAM scratch row-ranges (2048 rows each)
KC = DIM // P               # 4 k-chunks
MT = BLOCK // P             # 8 m-tiles
CG = 4                      # column groups of 2048
CGW = ROWS // CG            # 2048
NW = 512                    # matmul free width
TEMP_SCALE = 2.0            # 1/T
MAGIC = 0x5F3759DF


def _build_program():
    nc = bacc.Bacc(trn_type="TRN2")
    x_in = nc.declare_dram_parameter("x", [ROWS, DIM], F32, isOutput=False)
    logd_out = nc.declare_dram_parameter("logd", [P, MT], F32, isOutput=True)
    pos_out = nc.declare_dram_parameter("pos", [P, MT], F32, isOutput=True)

    with tile.TileContext(nc) as tc:
        with tc.tile_pool(name="xg", bufs=3) as xg_pool, \
             tc.tile_pool(name="zbig", bufs=3) as zbig_pool, \
             tc.tile_pool(name="sq", bufs=2) as sq_pool, \
             tc.tile_pool(name="small", bufs=2) as small_pool, \
             tc.tile_pool(name="single", bufs=1) as singles, \
             tc.tile_pool(name="zt", bufs=1) as zt_pool, \
             tc.tile_pool(name="escr", bufs=2) as e_pool, \
             tc.tile_pool(name="accp", bufs=2) as acc_pool, \
             tc.tile_pool(name="zdram", bufs=1, space="DRAM") as dram_pool, \
             tc.tile_pool(name="psum", bufs=2, space="PSUM") as psum_pool:

            n2 = singles.tile([P, NT], F32, tag="n2")
            inv = singles.tile([P, NT], F32, tag="inv")
            magic4 = singles.tile([P, TPG], I32, tag="magic4")
            nc.vector.memset(magic4, MAGIC)
            pos_acc = singles.tile([P, MT], F32, tag="pos_acc")
            den_all = singles.tile([P, MT], F32, tag="den_all")
            lnbias = singles.tile([P, 1], F32, tag="lnbias")
            nc.vector.memset(lnbias, -float(np.exp(2.0)))

            zd = [dram_pool.tile([ROWS // RG, DIM], BF16, tag=f"zd{r}", name=f"zd{r}")
                  for r in range(RG)]
            # zT[k][rg]: [128 (d-chunk k), 2048 (rows rg)] bf16
            zT = [[zt_pool.tile([P, CGW], BF16, tag=f"zt_{k}_{r}", name=f"zt_{k}_{r}")
                   for r in range(RG)] for k in range(KC)]

            zbigs = {}  # row-range idx -> assembled bf16 z tile [P, 16, DIM]

            # ---- Phase 1: normalize rows, write bf16 z to DRAM scratch ----
            for g in range(NG):
                r0 = g * TPG * P  # 512 rows per group
                xg = xg_pool.tile([P, TPG, DIM], F32, tag="xg")
                nc.sync.dma_start(
                    out=xg,
                    in_=x_in[r0:r0 + TPG * P, :].rearrange("(a p) d -> p a d", p=P))
                for a in range(TPG):
                    sq = sq_pool.tile([P, DIM], F32, tag="sq")
                    nc.vector.scalar_tensor_tensor(
                        out=sq, in0=xg[:, a, :], scalar=0.0, in1=xg[:, a, :],
                        op0=ALU.bypass, op1=ALU.mult,
                        accum_out=n2[:, g * TPG + a: g * TPG + a + 1])
                # rsqrt on this group's 4 norms: Quake seed + 2 Newton steps
                sl = n2[:, g * TPG:(g + 1) * TPG]
                isl = inv[:, g * TPG:(g + 1) * TPG]
                sh = small_pool.tile([P, TPG], I32, tag="sh")
                nc.vector.tensor_scalar(
                    out=sh, in0=sl.bitcast(I32), scalar1=1, scalar2=None,
                    op0=ALU.logical_shift_right)
                seed = small_pool.tile([P, TPG], I32, tag="seed")
                nc.vector.scalar_tensor_tensor(
                    out=seed, in0=magic4, scalar=0.0, in1=sh,
                    op0=ALU.bypass, op1=ALU.subtract)
                y = seed.bitcast(F32)
                for it in range(2):
                    ta = small_pool.tile([P, TPG], F32, tag="ta")
                    tb = small_pool.tile([P, TPG], F32, tag="tb")
                    nc.vector.tensor_mul(out=ta, in0=y, in1=y)
                    nc.vector.scalar_tensor_tensor(
                        out=tb, in0=ta, scalar=-0.5, in1=sl,
                        op0=ALU.mult, op1=ALU.mult)
                    nc.vector.tensor_scalar(
                        out=tb, in0=tb, scalar1=1.5, scalar2=None, op0=ALU.add)
                    dst = isl if it == 1 else y
                    nc.vector.tensor_mul(out=dst, in0=y, in1=tb)

                rr = g // 4
                if g % 4 == 0:
                    zbigs[rr] = zbig_pool.tile(
                        [P, 4 * TPG, DIM], BF16, tag="zbig", name=f"zbig{rr}")
                zb = zbigs[rr]
                jlo = (g % 4) * TPG
                for a in range(TPG):
                    nc.vector.tensor_scalar_mul(
                        out=zb[:, jlo + a, :], in0=xg[:, a, :],
                        scalar1=inv[:, g * TPG + a: g * TPG + a + 1])
                # positive pairs: rotated rows [0:1024) pair with [4096:5120)
                # i.e. tiles 0..7 (range 0 slices 0..7) with tiles 32..39
                # (range 2 slices 0..7)
                if g in (8, 9):
                    slo = (g - 8) * TPG
                    for a in range(TPG):
                        psc = sq_pool.tile([P, DIM], BF16, tag="psc")
                        nc.vector.scalar_tensor_tensor(
                            out=psc, in0=zbigs[0][:, slo + a, :], scalar=0.0,
                            in1=zb[:, jlo + a, :], op0=ALU.bypass, op1=ALU.mult,
                            accum_out=pos_acc[:, slo + a: slo + a + 1])
                if g % 4 == 3:
                    # single 2 MB writer per row-range: the downstream xbar
                    # transpose has very few sync-wait slots, so it must
                    # depend on exactly one DMA
                    nc.sync.dma_start(
                        out=zd[rr][:, :].rearrange("(s p) d -> p s d", p=P),
                        in_=zb)

            # ---- Phase 2: xbar transpose bf16 z -> zT ----
            for rr in range(RG):
                for k in range(KC):
                    nc.sync.dma_start_transpose(
                        out=zT[k][rr], in_=zd[rr][:, k * P:(k + 1) * P])

            # ---- Phase 3: row-block x all-columns matmul, exp row-sums ----
            for m in range(MT):
                accm = acc_pool.tile([P, CG], F32, tag="accm")
                for cg in range(CG):
                    ps = psum_pool.tile([P, CGW], F32, tag="ps")
                    for n in range(CGW // NW):
                        for k in range(KC):
                            nc.tensor.matmul(
                                ps[:, n * NW:(n + 1) * NW],
                                lhsT=zT[k][0][:, m * P:(m + 1) * P],
                                rhs=zT[k][cg][:, n * NW:(n + 1) * NW],
                                start=(k == 0), stop=(k == KC - 1))
                    e_scr = e_pool.tile([P, CGW], BF16, tag="escr")
                    nc.scalar.activation(
                        out=e_scr, in_=ps, func=AF.Exp, scale=TEMP_SCALE,
                        accum_out=accm[:, cg:cg + 1])
                nc.vector.reduce_sum(
                    out=den_all[:, m:m + 1], in_=accm,
                    axis=mybir.AxisListType.X)

            # ---- Phase 4: log-denoms, outputs ----
            logd = singles.tile([P, MT], F32, tag="logd")
            nc.scalar.activation(out=logd, in_=den_all, func=AF.Ln,
                                 bias=lnbias, scale=1.0)
            nc.sync.dma_start(out=logd_out[:, :], in_=logd)
            nc.sync.dma_start(out=pos_out[:, :], in_=pos_acc)

    nc.finalize()
    return nc


_CACHE = {}


def _run(full: np.ndarray, trace: bool = False, **kwargs):
    """Run the SPMD program on all 8 cores; returns BassKernelResults."""
    if "nc" not in _CACHE:
        _CACHE["nc"] = _build_program()
    nc = _CACHE["nc"]
    in_maps = [
        {"x": np.ascontiguousarray(np.roll(full, -BLOCK * c, axis=0))}
        for c in range(N_CORES)
    ]
    return run_bass_kernel_spmd(
        nc, in_maps, core_ids=list(range(N_CORES)), trace=trace, **kwargs)


def _merge(results) -> np.ndarray:
    logd_sum = 0.0
    pos_sum = 0.0
    for r in results:
        logd_sum += r["logd"].astype(np.float64).sum()
        pos_sum += r["pos"].astype(np.float64).sum()
    loss = (logd_sum - TEMP_SCALE * pos_sum) / (2.0 * BATCH)
    return np.array(loss, dtype=np.float32)


def kernel(emb_i: np.ndarray, emb_j: np.ndarray) -> np.ndarray:
    full = np.concatenate(
        [np.asarray(emb_i, np.float32), np.asarray(emb_j, np.float32)], axis=0)
    return _merge(_run(full).results)



# revision 7
# speedup vs baseline: 1.7159x; 1.7159x over previous
"""Trainium2 Bass kernel for SimCLR NT-Xent contrastive loss (N=4096, D=512, T=0.5).

Math: with z = rownorm(concat(emb_i, emb_j)) (8192x512) and S = z @ z.T:
  loss = (1/2N) * [ sum_r log(rowsum_r(exp(S/T)) - exp(1/T)) - (1/T) * sum_r S[r, (r+N) mod 2N] ]

Distribution: data-parallel over rows of z. Each of the 8 cores receives a
block-rotated copy of the concatenated input (rotation by 1024*c rows), so the
same program computes row block [0:1024) of its rotated similarity matrix
against all 8192 columns. Rotation preserves both the row set and the +N pair
structure (mod 2N).

v2 design (vs v1 which serialized phases and roundtripped z via DRAM):
  - z is scaled by 16/||x|| and quantized to fp8e4 (E4M3); the matmul runs in
    DoubleRow perf mode (K=256 per instr -> 2x PE throughput). exp() applies
    scale 2/256 to undo the 16x16 factor. E4M3's ~4% element quantization
    averages out across 512-d dots (S abs err ~2e-3) - far inside the 2e-2 gate.
  - transpose happens on-chip: PE transpose (identity matmul) of bf16 z tiles
    into PSUM, then GPSIMD copy-casts PSUM->SBUF fp8 zT. No DRAM roundtrip:
    HBM traffic drops from 32MB to 16MB per core.
  - software-pipelined program order: prep(r0), prep(r1), mm(r0), prep(r2),
    mm(r1), prep(r3), mm(r2), mm(r3) so every engine's in-order queue stays
    busy (loads/normalize of range k+1 overlap matmul+exp of range k).
  - engine balance: row-norm squares split ACT/DVE ~1/3-2/3, scaling on DVE,
    exp+ln on ACT (all ACT funcs live in one HW table -> no table thrash),
    PSUM->SBUF casts on GPSIMD.
Host merges 8 partial [128,8] tensors (log-denoms, 256x pair-dots) into the
scalar loss.
"""

import numpy as np

for _p in ("/opt/trn_rl_repo", "/root/.axon_site/_ro/trn_rl_repo"):
    try:
        import concourse  # noqa: F401
        break
    except ImportError:
        import sys
        if _p not in sys.path:
            sys.path.insert(0, _p)

import concourse.bass as bass
import concourse.bacc as bacc
import concourse.tile as tile
from concourse import mybir
from concourse.bass_utils import run_bass_kernel_spmd
from concourse.masks import make_identity

F32 = mybir.dt.float32
I32 = mybir.dt.int32
BF16 = mybir.dt.bfloat16
FP8 = mybir.dt.float8e4
ALU = mybir.AluOpType
AF = mybir.ActivationFunctionType
PM = mybir.MatmulPerfMode

N_CORES = 8
BATCH = 4096
DIM = 512
ROWS = 2 * BATCH            # 8192
BLOCK = ROWS // N_CORES     # 1024 rows per core
P = 128                     # partitions
NT = ROWS // P              # 64 row tiles
RG = 4                      # 4 column ranges of 2048 rows
TPR = NT // RG              # 16 row-tiles per range
KC = DIM // P               # 4 k-chunks
MT = BLOCK // P             # 8 m-tiles
ZSCALE = 16.0               # fp8 pre-scale; exp scale divides by 16^2
EXPSCALE = 2.0 / (ZSCALE * ZSCALE)
MAGIC = 0x5F3759DF


def _build_program():
    nc = bacc.Bacc(trn_type="TRN2")
    x_in = nc.declare_dram_parameter("x", [ROWS, DIM], F32, isOutput=False)
    logd_out = nc.declare_dram_parameter("logd", [P, MT], F32, isOutput=True)
    pos_out = nc.declare_dram_parameter("pos", [P, MT], F32, isOutput=True)

    with tile.TileContext(nc) as tc:
        with tc.tile_pool(name="xg", bufs=6) as xg_pool, \
             tc.tile_pool(name="zstage", bufs=4) as z_pool, \
             tc.tile_pool(name="sqd", bufs=2) as sqd_pool, \
             tc.tile_pool(name="small", bufs=2) as small_pool, \
             tc.tile_pool(name="edump", bufs=3) as e_pool, \
             tc.tile_pool(name="single", bufs=1) as singles, \
             tc.tile_pool(name="psum_mm", bufs=3, space="PSUM") as psmm_pool, \
             tc.tile_pool(name="psum_tr", bufs=2, space="PSUM") as pstr_pool:

            n2 = singles.tile([P, NT], F32, tag="n2")
            inv = singles.tile([P, NT], F32, tag="inv")
            magic16 = singles.tile([P, TPR], I32, tag="magic16")
            nc.vector.memset(magic16, MAGIC)
            pos_acc = singles.tile([P, MT], F32, tag="pos_acc")
            accm = singles.tile([P, MT, 2 * RG], F32, tag="accm")
            den_all = singles.tile([P, MT], F32, tag="den_all")
            lnbias = singles.tile([P, 1], F32, tag="lnbias")
            nc.vector.memset(lnbias, -float(np.exp(2.0)))
            ident = singles.tile([P, P], BF16, tag="ident")
            make_identity(nc, ident)

            # zT[rr]: [128 (d within chunk), 4 (k-chunk), 2048 (rows)] fp8
            zT = [singles.tile([P, KC, 2 * BLOCK], FP8, tag=f"zt{r}",
                               name=f"zt{r}") for r in range(RG)]
            # bf16 z rows kept for the positive-pair dot: ranges 0 and 2
            zkeep = {rr: singles.tile([P, TPR, DIM], BF16, tag=f"zkeep{rr}",
                                      name=f"zkeep{rr}") for rr in (0, 2)}

            def prep_range(rr):
                """load 2048 rows, rownorm, scale*16 -> bf16, PE-transpose,
                copy-cast to fp8 zT[rr]."""
                xgs = []
                for g in range(4):          # 4 groups of 512 rows
                    r0 = rr * 2 * BLOCK + g * 4 * P
                    xg = xg_pool.tile([P, 4, DIM], F32, tag="xg")
                    nc.sync.dma_start(
                        out=xg,
                        in_=x_in[r0:r0 + 4 * P, :].rearrange(
                            "(a p) d -> p a d", p=P))
                    xgs.append(xg)
                    for a in range(4):
                        t = rr * TPR + g * 4 + a
                        # square+rowsum, split ACT/DVE
                        if t % 2 == 0:
                            sqa = sqd_pool.tile([P, DIM], BF16, tag="sqa")
                            nc.scalar.activation(
                                out=sqa, in_=xg[:, a, :], func=AF.Square,
                                accum_out=n2[:, t:t + 1])
                        else:
                            sqd = sqd_pool.tile([P, DIM], F32, tag="sqd")
                            nc.vector.scalar_tensor_tensor(
                                out=sqd, in0=xg[:, a, :], scalar=0.0,
                                in1=xg[:, a, :], op0=ALU.bypass, op1=ALU.mult,
                                accum_out=n2[:, t:t + 1])
                # rsqrt via Quake seed + 2 Newton steps on [P, 16], then *16
                sl = n2[:, rr * TPR:(rr + 1) * TPR]
                isl = inv[:, rr * TPR:(rr + 1) * TPR]
                sh = small_pool.tile([P, TPR], I32, tag="sh")
                nc.vector.tensor_scalar(
                    out=sh, in0=sl.bitcast(I32), scalar1=1, scalar2=None,
                    op0=ALU.logical_shift_right)
                seed = small_pool.tile([P, TPR], I32, tag="seed")
                nc.vector.scalar_tensor_tensor(
                    out=seed, in0=magic16, scalar=0.0, in1=sh,
                    op0=ALU.bypass, op1=ALU.subtract)
                y = seed.bitcast(F32)
                for it in range(2):
                    ta = small_pool.tile([P, TPR], F32, tag="ta")
                    tb = small_pool.tile([P, TPR], F32, tag="tb")
                    nc.vector.tensor_mul(out=ta, in0=y, in1=y)
                    nc.vector.scalar_tensor_tensor(
                        out=tb, in0=ta, scalar=-0.5, in1=sl,
                        op0=ALU.mult, op1=ALU.mult)
                    nc.vector.tensor_scalar(
                        out=tb, in0=tb, scalar1=1.5, scalar2=None, op0=ALU.add)
                    if it == 0:
                        nc.vector.tensor_mul(out=y, in0=y, in1=tb)
                    else:
                        # fold the fp8 pre-scale: inv = 16 * rsqrt(n2)
                        nc.vector.scalar_tensor_tensor(
                            out=isl, in0=y, scalar=ZSCALE, in1=tb,
                            op0=ALU.mult, op1=ALU.mult)

                # scale rows to bf16 (16/||x||), PE-transpose pairs of tiles,
                # copy-cast PSUM bf16 -> SBUF fp8 zT (split ACT/DVE)
                tp = None
                for g in range(4):
                    for a in range(4):
                        t = rr * TPR + g * 4 + a
                        c = g * 4 + a       # tile idx within range
                        if rr in zkeep:
                            zb = zkeep[rr][:, c, :]
                        else:
                            zb = z_pool.tile([P, DIM], BF16, tag="zb")
                        nc.vector.tensor_scalar_mul(
                            out=zb, in0=xgs[g][:, a, :],
                            scalar1=inv[:, t:t + 1])
                        half = c % 2
                        if half == 0:
                            tp = pstr_pool.tile([P, KC, 2 * P], BF16, tag="tp")
                        for k in range(KC):
                            nc.tensor.transpose(
                                tp[:, k, half * P:(half + 1) * P],
                                zb[:, k * P:(k + 1) * P], ident)
                        if half == 1:
                            # two tiles' transposes -> zT cols [c-1, c+1)*128
                            dst = zT[rr][:, :, (c - 1) * P:(c + 1) * P]
                            if c % 4 == 1:
                                nc.scalar.activation(
                                    out=dst, in_=tp, func=AF.Copy)
                            else:
                                nc.vector.tensor_copy(out=dst, in_=tp)

            def mm_range(rr):
                """row-block [0:1024) x columns of range rr: fp8 DoubleRow
                matmul + exp row-sum accumulation."""
                for m in range(MT):
                    for h in range(2):      # two 1024-col chunks
                        ps = psmm_pool.tile([P, 2 * DIM], F32, tag="ps")
                        for kp in range(2):  # k-pairs (DoubleRow: K=256)
                            for n in range(2):
                                nc.tensor.matmul(
                                    ps[:, n * DIM:(n + 1) * DIM],
                                    lhsT=zT[0][:, 2 * kp:2 * kp + 2,
                                               m * P:(m + 1) * P],
                                    rhs=zT[rr][:, 2 * kp:2 * kp + 2,
                                               h * 2 * DIM + n * DIM:
                                               h * 2 * DIM + (n + 1) * DIM],
                                    start=(kp == 0), stop=(kp == 1),
                                    perf_mode=PM.DoubleRow)
                        e_scr = e_pool.tile([P, 2 * DIM], BF16, tag="escr")
                        nc.scalar.activation(
                            out=e_scr, in_=ps, func=AF.Exp, scale=EXPSCALE,
                            accum_out=accm[:, m, 2 * rr + h:2 * rr + h + 1])

            def pos_block():
                """positive pairs: rows [0:1024) dot rows [4096:5120),
                via bf16 z (scaled by 16 -> pos accumulates 256x)."""
                for c in range(MT):
                    psc = sqd_pool.tile([P, DIM], BF16, tag="psc")
                    nc.vector.scalar_tensor_tensor(
                        out=psc, in0=zkeep[0][:, c, :], scalar=0.0,
                        in1=zkeep[2][:, c, :], op0=ALU.bypass, op1=ALU.mult,
                        accum_out=pos_acc[:, c:c + 1])

            # software pipeline
            prep_range(0)
            prep_range(1)
            mm_range(0)
            prep_range(2)
            mm_range(1)
            prep_range(3)
            pos_block()
            mm_range(2)
            mm_range(3)

            # log-denoms: den = sum of the 8 per-range exp row-sums
            logd = singles.tile([P, MT], F32, tag="logd")
            for m in range(MT):
                nc.vector.reduce_sum(
                    out=den_all[:, m:m + 1], in_=accm[:, m, :],
                    axis=mybir.AxisListType.X)
            nc.scalar.activation(out=logd, in_=den_all, func=AF.Ln,
                                 bias=lnbias, scale=1.0)
            nc.sync.dma_start(out=logd_out[:, :], in_=logd)
            nc.sync.dma_start(out=pos_out[:, :], in_=pos_acc)

    nc.finalize()
    return nc


_CACHE = {}


def _run(full: np.ndarray, trace: bool = False, **kwargs):
    """Run the SPMD program on all 8 cores; returns BassKernelResults."""
    if "nc" not in _CACHE:
        _CACHE["nc"] = _build_program()
    nc = _CACHE["nc"]
    in_maps = [
        {"x": np.ascontiguousarray(np.roll(full, -BLOCK * c, axis=0))}
        for c in range(N_CORES)
    ]
    return run_bass_kernel_spmd(
        nc, in_maps, core_ids=list(range(N_CORES)), trace=trace, **kwargs)


def _merge(results) -> np.ndarray:
    logd_sum = 0.0
    pos_sum = 0.0
    for r in results:
        logd_sum += r["logd"].astype(np.float64).sum()
        pos_sum += r["pos"].astype(np.float64).sum()
    # pos accumulated (16*z_i)·(16*z_j) = 256x; temperature scale 1/T = 2
    loss = (logd_sum - 2.0 * pos_sum / (ZSCALE * ZSCALE)) / (2.0 * BATCH)
    return np.array(loss, dtype=np.float32)


def kernel(emb_i: np.ndarray, emb_j: np.ndarray) -> np.ndarray:
    full = np.concatenate(
        [np.asarray(emb_i, np.float32), np.asarray(emb_j, np.float32)], axis=0)
    return _merge(_run(full).results)


# revision 8
# speedup vs baseline: 2.2996x; 1.3401x over previous
"""Trainium2 Bass kernel for SimCLR NT-Xent contrastive loss (N=4096, D=512, T=0.5).

Math: with z = rownorm(concat(emb_i, emb_j)) (8192x512) and S = z @ z.T:
  loss = (1/2N) * [ sum_r log(rowsum_r(exp(S/T)) - exp(1/T)) - (1/T) * sum_r S[r, (r+N) mod 2N] ]

Distribution (v3, symmetric): each of the 8 cores gets a block-rotated copy of
the input (rotation by 1024*c rows). Core c computes the 1024x1024 similarity
blocks (0, d) for d = 0..4 in its rotated frame (= global blocks (c, c+d)).
Because S is symmetric, global block (b, b+k) for k in 5,6,7 equals the
transpose of block (b+k, b+k + (8-k)) computed by core b+k with d = 8-k in
1..3. So:
  - every core accumulates exp-ROW-sums for its d = 0..4 blocks (-> den)
  - every core also accumulates per-partition exp-COLUMN-sums for d = 1..3
    (-> csum, finished on the host by summing over partitions)
The host assembles full denominators: den[b] + csum from cores b-1, b-2, b-3.
Matmul/exp work drops to 5/8 of the full row-block approach.

Per-core mechanics:
  - fp8 (E4M3) z, scaled by 16: DoubleRow matmul (K=256/instr, 2x PE rate);
    exp applies 2/256 scale. Quantization noise on the 512-d dots is ~2e-3
    (vs the 2e-2 gate).
  - on-chip transpose: PE identity-matmul of bf16 z tiles -> PSUM, copy-cast
    to fp8 zT in SBUF (split ACT/DVE). No DRAM roundtrip: HBM traffic is just
    the 10MB of input rows the core actually needs.
  - engine balance: squares+rowsum on DVE, scale-to-bf16 on ACT (Copy with
    per-partition scale AP), exp on ACT, csum adds on DVE, rsqrt via Quake
    iteration on DVE. All ACT funcs (Square/Copy/Exp) share one HW table.
  - software-pipelined program order over the 5 column blocks.
"""

import numpy as np

for _p in ("/opt/trn_rl_repo", "/root/.axon_site/_ro/trn_rl_repo"):
    try:
        import concourse  # noqa: F401
        break
    except ImportError:
        import sys
        if _p not in sys.path:
            sys.path.insert(0, _p)

import concourse.bass as bass
import concourse.bacc as bacc
import concourse.tile as tile
from concourse import mybir
from concourse.bass_utils import run_bass_kernel_spmd
from concourse.masks import make_identity

F32 = mybir.dt.float32
I32 = mybir.dt.int32
BF16 = mybir.dt.bfloat16
FP8 = mybir.dt.float8e4
ALU = mybir.AluOpType
AF = mybir.ActivationFunctionType
PM = mybir.MatmulPerfMode

N_CORES = 8
BATCH = 4096
DIM = 512
ROWS = 2 * BATCH            # 8192
BLOCK = ROWS // N_CORES     # 1024 rows per core
P = 128                     # partitions
KC = DIM // P               # 4 k-chunks
MT = BLOCK // P             # 8 m-tiles (also tiles per 1024-row block)
DB = 5                      # d-blocks computed per core (0..4)
NTT = DB * MT               # 40 row tiles loaded per core
ZSCALE = 16.0               # fp8 pre-scale; exp scale divides by 16^2
EXPSCALE = 2.0 / (ZSCALE * ZSCALE)
MAGIC = 0x5F3759DF


def _build_program():
    nc = bacc.Bacc(trn_type="TRN2")
    x_in = nc.declare_dram_parameter("x", [ROWS, DIM], F32, isOutput=False)
    den_out = nc.declare_dram_parameter("den", [P, MT], F32, isOutput=True)
    pos_out = nc.declare_dram_parameter("pos", [P, MT], F32, isOutput=True)
    csum_out = nc.declare_dram_parameter("csum", [P, 3, BLOCK], BF16,
                                         isOutput=True)

    with tile.TileContext(nc) as tc:
        with tc.tile_pool(name="xg", bufs=4) as xg_pool, \
             tc.tile_pool(name="zstage", bufs=4) as z_pool, \
             tc.tile_pool(name="sqd", bufs=2) as sqd_pool, \
             tc.tile_pool(name="small", bufs=2) as small_pool, \
             tc.tile_pool(name="edump", bufs=3) as e_pool, \
             tc.tile_pool(name="single", bufs=1) as singles, \
             tc.tile_pool(name="psum_mm", bufs=3, space="PSUM") as psmm_pool, \
             tc.tile_pool(name="psum_tr", bufs=2, space="PSUM") as pstr_pool:

            n2 = singles.tile([P, NTT], F32, tag="n2")
            inv = singles.tile([P, NTT], F32, tag="inv")
            magic8 = singles.tile([P, MT], I32, tag="magic8")
            nc.vector.memset(magic8, MAGIC)
            pos_acc = singles.tile([P, MT], F32, tag="pos_acc")
            accm = singles.tile([P, MT, DB], F32, tag="accm")
            den_all = singles.tile([P, MT], F32, tag="den_all")
            ident = singles.tile([P, P], BF16, tag="ident")
            make_identity(nc, ident)
            csum = [singles.tile([P, BLOCK], BF16, tag=f"cs{d}",
                                 name=f"cs{d}") for d in range(3)]
            for d in range(3):
                nc.vector.memset(csum[d], 0.0)

            # zT[d]: [128 (d within chunk), 4 (k-chunk), 1024 (rows)] fp8
            zT = [singles.tile([P, KC, BLOCK], FP8, tag=f"zt{d}",
                               name=f"zt{d}") for d in range(DB)]
            # bf16 z rows kept for the positive-pair dot: blocks 0 and 4
            zkeep = {d: singles.tile([P, MT, DIM], BF16, tag=f"zkeep{d}",
                                     name=f"zkeep{d}") for d in (0, 4)}

            cp_idx = [0]    # running pair-copy index for ACT/DVE split

            def prep_block(d):
                """load 1024 rows, rownorm, scale*16 -> bf16, PE-transpose,
                copy-cast to fp8 zT[d]."""
                xgs = []
                for g in range(2):          # 2 groups of 512 rows
                    r0 = d * BLOCK + g * 4 * P
                    xg = xg_pool.tile([P, 4, DIM], F32, tag="xg")
                    nc.sync.dma_start(
                        out=xg,
                        in_=x_in[r0:r0 + 4 * P, :].rearrange(
                            "(a p) d -> p a d", p=P))
                    xgs.append(xg)
                    for a in range(4):
                        t = d * MT + g * 4 + a
                        sqd = sqd_pool.tile([P, DIM], F32, tag="sqd")
                        nc.vector.scalar_tensor_tensor(
                            out=sqd, in0=xg[:, a, :], scalar=0.0,
                            in1=xg[:, a, :], op0=ALU.bypass, op1=ALU.mult,
                            accum_out=n2[:, t:t + 1])
                # rsqrt via Quake seed + 2 Newton steps on [P, 8], then *16
                sl = n2[:, d * MT:(d + 1) * MT]
                isl = inv[:, d * MT:(d + 1) * MT]
                sh = small_pool.tile([P, MT], I32, tag="sh")
                nc.vector.tensor_scalar(
                    out=sh, in0=sl.bitcast(I32), scalar1=1, scalar2=None,
                    op0=ALU.logical_shift_right)
                seed = small_pool.tile([P, MT], I32, tag="seed")
                nc.vector.scalar_tensor_tensor(
                    out=seed, in0=magic8, scalar=0.0, in1=sh,
                    op0=ALU.bypass, op1=ALU.subtract)
                y = seed.bitcast(F32)
                for it in range(2):
                    ta = small_pool.tile([P, MT], F32, tag="ta")
                    tb = small_pool.tile([P, MT], F32, tag="tb")
                    nc.vector.tensor_mul(out=ta, in0=y, in1=y)
                    nc.vector.scalar_tensor_tensor(
                        out=tb, in0=ta, scalar=-0.5, in1=sl,
                        op0=ALU.mult, op1=ALU.mult)
                    nc.vector.tensor_scalar(
                        out=tb, in0=tb, scalar1=1.5, scalar2=None, op0=ALU.add)
                    if it == 0:
                        nc.vector.tensor_mul(out=y, in0=y, in1=tb)
                    else:
                        # fold the fp8 pre-scale: inv = 16 * rsqrt(n2)
                        nc.vector.scalar_tensor_tensor(
                            out=isl, in0=y, scalar=ZSCALE, in1=tb,
                            op0=ALU.mult, op1=ALU.mult)

                # scale rows to bf16 on ACT (Copy, per-partition scale AP),
                # PE-transpose pairs, copy-cast PSUM bf16 -> SBUF fp8 zT
                tp = None
                for c in range(MT):
                    t = d * MT + c
                    if d in zkeep:
                        zb = zkeep[d][:, c, :]
                    else:
                        zb = z_pool.tile([P, DIM], BF16, tag="zb")
                    nc.scalar.activation(
                        out=zb, in_=xgs[c // 4][:, c % 4, :], func=AF.Copy,
                        scale=inv[:, t:t + 1])
                    half = c % 2
                    if half == 0:
                        tp = pstr_pool.tile([P, KC, 2 * P], BF16, tag="tp")
                    for k in range(KC):
                        nc.tensor.transpose(
                            tp[:, k, half * P:(half + 1) * P],
                            zb[:, k * P:(k + 1) * P], ident)
                    if half == 1:
                        dst = zT[d][:, :, (c - 1) * P:(c + 1) * P]
                        if cp_idx[0] % 5 < 2:
                            nc.scalar.activation(out=dst, in_=tp, func=AF.Copy)
                        else:
                            nc.vector.tensor_copy(out=dst, in_=tp)
                        cp_idx[0] += 1

            def mm_block(d):
                """rows [0:1024) x columns of block d: fp8 DoubleRow matmul,
                exp row-sums; for d=1..3 also per-partition column sums."""
                for m in range(MT):
                    ps = psmm_pool.tile([P, BLOCK], F32, tag="ps")
                    for kp in range(2):     # k-pairs (DoubleRow: K=256)
                        for n in range(2):
                            nc.tensor.matmul(
                                ps[:, n * DIM:(n + 1) * DIM],
                                lhsT=zT[0][:, 2 * kp:2 * kp + 2,
                                           m * P:(m + 1) * P],
                                rhs=zT[d][:, 2 * kp:2 * kp + 2,
                                          n * DIM:(n + 1) * DIM],
                                start=(kp == 0), stop=(kp == 1),
                                perf_mode=PM.DoubleRow)
                    e_scr = e_pool.tile([P, BLOCK], BF16, tag="escr")
                    nc.scalar.activation(
                        out=e_scr, in_=ps, func=AF.Exp, scale=EXPSCALE,
                        accum_out=accm[:, m, d:d + 1])
                    if 1 <= d <= 3:
                        nc.vector.tensor_tensor(
                            out=csum[d - 1], in0=e_scr, in1=csum[d - 1],
                            op=ALU.add)

            def pos_block():
                """positive pairs: rotated rows [0:1024) dot rows
                [4096:5120) elementwise (z scaled 16x -> pos is 256x)."""
                for c in range(MT):
                    psc = sqd_pool.tile([P, DIM], BF16, tag="psc")
                    nc.vector.scalar_tensor_tensor(
                        out=psc, in0=zkeep[0][:, c, :], scalar=0.0,
                        in1=zkeep[4][:, c, :], op0=ALU.bypass, op1=ALU.mult,
                        accum_out=pos_acc[:, c:c + 1])

            # software pipeline over the 5 blocks
            prep_block(0)
            prep_block(1)
            mm_block(0)
            prep_block(2)
            mm_block(1)
            prep_block(3)
            mm_block(2)
            prep_block(4)
            pos_block()
            mm_block(3)
            mm_block(4)

            # den = sum of the 5 per-block exp row-sums (raw; host does log)
            for m in range(MT):
                nc.vector.reduce_sum(
                    out=den_all[:, m:m + 1], in_=accm[:, m, :],
                    axis=mybir.AxisListType.X)
            nc.sync.dma_start(out=den_out[:, :], in_=den_all)
            nc.sync.dma_start(out=pos_out[:, :], in_=pos_acc)
            for d in range(3):
                nc.sync.dma_start(out=csum_out[:, d, :], in_=csum[d])

    nc.finalize()
    return nc


_CACHE = {}


def _run(full: np.ndarray, trace: bool = False, **kwargs):
    """Run the SPMD program on all 8 cores; returns BassKernelResults."""
    if "nc" not in _CACHE:
        _CACHE["nc"] = _build_program()
    nc = _CACHE["nc"]
    in_maps = [
        {"x": np.ascontiguousarray(np.roll(full, -BLOCK * c, axis=0))}
        for c in range(N_CORES)
    ]
    return run_bass_kernel_spmd(
        nc, in_maps, core_ids=list(range(N_CORES)), trace=trace, **kwargs)


def _merge(results) -> np.ndarray:
    # rowsum partials: den[c][p, m] = rotated row 128m+p of core c
    # csum partials: csum[c][p, d-1, j] = sum over rows {128m+p} of
    #   exp-block (0, d); host finishes the partition sum.
    den_full = np.zeros(ROWS, dtype=np.float64)
    pos_sum = 0.0
    for c, r in enumerate(results):
        den = r["den"].astype(np.float64)           # [128, 8]
        j = np.arange(BLOCK)
        den_full[BLOCK * c + j] += den[j % P, j // P]
        cs = r["csum"].astype(np.float64).sum(axis=0)   # [3, 1024]
        for d in (1, 2, 3):
            b = (c + d) % N_CORES
            den_full[BLOCK * b + j] += cs[d - 1]
        pos_sum += r["pos"].astype(np.float64).sum()
    # drop the diagonal exp(2*||zq||^2) ~ e^2; pos accumulated 256x
    logd_sum = np.log(den_full - np.exp(2.0)).sum()
    loss = (logd_sum - 2.0 * pos_sum / (ZSCALE * ZSCALE)) / (2.0 * BATCH)
    return np.array(loss, dtype=np.float32)


def kernel(emb_i: np.ndarray, emb_j: np.ndarray) -> np.ndarray:
    full = np.concatenate(
        [np.asarray(emb_i, np.float32), np.asarray(emb_j, np.float32)], axis=0)
    return _merge(_run(full).results)
